# revision 20
# baseline (speedup 1.0000x reference)
# Trainium2 Bass kernel for nn_ChannelAttentionBlock:
#   per batch b: F = x[b].reshape(4096, 128)  (raw row-major view)
#                A = F @ F.T            [4096, 4096]
#                P = softmax(A, axis=-1)
#                out[b] = (F.T @ P).reshape(128, 64, 64)
#
# Sharding: data-parallel over batch — B=8 batches, one per NeuronCore.
#
# Per-core algorithm (X := F.T as [128, 4096], n-blocked by 128 rows of A):
#   prologue: DMA F row-blocks (contiguous) -> XT tiles; PE-transpose them
#             into Xr = float32r(F.T); negdiag[n] = -sum_k F[n,k]^2 (softmax
#             shift — any per-row constant is mathematically exact; the
#             diagonal dominates each row of A for this Gram matrix, making
#             exp overflow-safe).
#   per row-block i: A_i = Xr[:,blk_i].T @ Xr (8 matmuls of [128,512] into
#             chunks of [1536,1536,1024]), P~_i = exp(A_i + negdiag_i) via
#             ScalarE with per-partition bias and accum_out giving the row
#             sums; fold 1/s into the O-matmul's stationary operand:
#             Xs_i = XT_i * (1/s_i); O += Xs_i.T @ P~_i, PSUM-accumulated
#             over groups of 4 blocks through one transient 2-bank slot,
#             then VectorE-added into an SBUF accumulator.
#
# The loop is software-pipelined for the ScalarE exp stream (the bottleneck
# engine at ~145us busy): A-chunks own two dedicated 3-bank PSUM slots, and
# each O column-group burst is emitted between a_phases so the in-order PE
# never starves ACT. TimelineSim predicts ~168us/core; ScalarE ~86% busy.
#
# Matmuls run in float32r (TF32-like, 1 cycle/row at N=512 — 4x faster than
# plain fp32, ~16x more accurate than bf16; measured rel err ~1.5e-4).

import numpy as np

import concourse.bass as bass
import concourse.mybir as mybir
import concourse.tile as tile
from concourse.bass_utils import run_bass_kernel_spmd

N_CORES = 8
D = 128          # feature dim / partition dim
N = 4096         # sequence dim (64*64)
NB = N // 128    # 32 row blocks
F32 = mybir.dt.float32
F32R = mybir.dt.float32r
AX = mybir.AxisListType
ALU = mybir.AluOpType
ACT = mybir.ActivationFunctionType


def _split_waits(nc, max_waits=1):
    """walrus in this toolchain encodes at most 1 semaphore wait per
    instruction; Tile emits several on its tail drain. Move overflow waits
    onto preceding same-engine NoOps (sequencer executes them in order)."""
    n_split = 0
    for f in nc.m.functions:
        for bb in f.blocks:
            new_insts = []
            for inst in bb.instructions:
                si = inst.sync_info
                if si is not None and si.on_wait and len(si.on_wait) > max_waits:
                    waits = list(si.on_wait)
                    chunks = [waits[i:i + max_waits]
                              for i in range(0, len(waits), max_waits)]
                    for chunk in chunks[:-1]:
                        nop = mybir.InstNoOp(
                            name=nc.get_next_instruction_name(), ins=[], outs=[])
                        nop.engine = inst.engine
                        nop.sync_info = mybir.SyncInfo(on_wait=chunk, on_update=[])
                        new_insts.append(nop)
                        n_split += 1
                    inst.sync_info = mybir.SyncInfo(
                        on_wait=chunks[-1],
                        on_update=list(si.on_update) if si.on_update else [])
                new_insts.append(inst)
            bb.instructions = new_insts
    return n_split


def _build_nc():
    nc = bass.Bass("TRN2", target_bir_lowering=False, debug=False)
    x_d = nc.dram_tensor("x", [N, D], F32, kind="ExternalInput").ap()
    y_d = nc.dram_tensor("y", [D, N], F32, kind="ExternalOutput").ap()

    # A-row chunk widths per block (sum = N). Fewer/bigger chunks amortize
    # the ScalarE per-instruction overhead (~404ns: access-latency + accum
    # read + dispatch) over more exp elements.
    CH = [1536, 1536, 1024]
    CO = [0, 1536, 3072]          # chunk column offsets
    GRP = 4                       # blocks per O-accumulation group

    with tile.TileContext(nc) as tc:
        with tc.tile_pool(name="const", bufs=1) as const, \
             tc.tile_pool(name="ppool", bufs=1) as ppool, \
             tc.tile_pool(name="loop", bufs=12) as loop, \
             tc.tile_pool(name="trans", bufs=1, space="PSUM") as trans_pool:

            XT = const.tile([D, N], F32, tag="XT")      # XT[:,128i:..] = F[blk_i,:]
            Xr = const.tile([D, N], F32R, tag="Xr")     # rounded F.T
            O_acc = const.tile([D, N], F32, tag="Oacc") # SBUF output accumulator
            negdiag = const.tile([D, NB], F32, tag="negdiag")
            ident = const.tile([D, D], F32, tag="ident")

            # XT[p, 128i+k] = x_d[128i+p, k]; contiguous 512B bursts per row.
            # The transpose identity is built on-chip (memset + affine_select)
            # so the HWDGE dispatch queue only carries the x loads.
            nc.gpsimd.memset(ident[:], 1.0)
            nc.gpsimd.affine_select(ident[:], ident[:], [[1, D]],
                                    ALU.is_equal, 0.0, base=0,
                                    channel_multiplier=-1)
            # Input DMAs split across both HWDGE queues (SP + ACT
            # sequencers) to halve the serialized queue-dispatch time.
            x_r = x_d.rearrange("(i p) k -> p i k", p=D)
            XT_v = XT[:].rearrange("p (i k) -> p i k", k=D)
            for g in range(8):
                eng = nc.sync if g % 2 == 0 else nc.scalar
                eng.dma_start(XT_v[:, g * 4:(g + 1) * 4, :],
                              x_r[:, g * 4:(g + 1) * 4, :])

            # Prologue: Xr = f32r(F.T) via PE transposes, 4 per PSUM tile,
            # one VectorE evacuation copy each (kept off ScalarE so the
            # in-order exp stream is not stalled behind prologue copies).
            def xr_group(g, split=1):
                for v in range(split):
                    w = 512 // split
                    tp = trans_pool.tile([D, w], F32, tag="ta", bufs=2)
                    for u in range(w // D):
                        i = (g * 512 + v * w) // D + u
                        nc.tensor.transpose(tp[:, u * D:(u + 1) * D],
                                            XT[:, i * D:(i + 1) * D], ident[:])
                    nc.vector.tensor_copy(
                        Xr[:, g * 512 + v * w:g * 512 + (v + 1) * w], tp[:])

            # Software-pipelined main loop. A-chunks own two dedicated
            # 3-bank PSUM slots (tag "ta"); all O columns flow through ONE
            # transient 2-bank slot (tag "tot") that PSUM-accumulates GRP
            # blocks per VectorE add into the SBUF accumulator.
            state = {}

            def a_phase(i):
                lhsA = Xr[:, i * D:(i + 1) * D]
                # negdiag[p, i] = -sum_k F[128i+p, k]^2 (= -A[n,n], n=128i+p)
                nsq = loop.tile([D, D], F32, tag="nsq")
                nc.vector.scalar_tensor_tensor(
                    nsq[:], XT[:, i * D:(i + 1) * D], -1.0,
                    XT[:, i * D:(i + 1) * D], op0=ALU.mult, op1=ALU.mult)
                nc.vector.tensor_reduce(negdiag[:, i:i + 1], nsq[:],
                                        axis=AX.X, op=ALU.add)
                spart = loop.tile([D, 3], F32, tag="spart")
                pchunks = []
                for c, (w, o) in enumerate(zip(CH, CO)):
                    At = trans_pool.tile([D, w], F32, tag="ta", bufs=2)
                    for m in range(w // 512):
                        nc.tensor.matmul(
                            At[:, m * 512:(m + 1) * 512], lhsA,
                            Xr[:, o + m * 512:o + (m + 1) * 512],
                            start=True, stop=True)
                    P_c = ppool.tile([D, w], F32R, tag=f"p{c}", bufs=9)
                    nc.scalar.activation(P_c[:], At[:], ACT.Exp,
                                         bias=negdiag[:, i:i + 1],
                                         accum_out=spart[:, c:c + 1])
                    pchunks.append(P_c)
                state[i] = (pchunks, spart)

            def s_chain(i):
                pchunks, spart = state.pop(i)
                s = loop.tile([D, 1], F32, tag="s")
                nc.vector.tensor_reduce(s[:], spart[:], axis=AX.X, op=ALU.add)
                r = loop.tile([D, 1], F32, tag="r")
                nc.vector.reciprocal(r[:], s[:])
                Xs = loop.tile([D, D], F32R, tag="xs")
                nc.gpsimd.tensor_scalar_mul(Xs[:], XT[:, i * D:(i + 1) * D], r[:])
                return pchunks, Xs

            def p_slice(pchunks, lo, hi):
                """AP for O-columns [lo, hi) inside the per-block chunk list."""
                for c, (w, o) in enumerate(zip(CH, CO)):
                    if o <= lo and hi <= o + w:
                        return pchunks[c][:, lo - o:hi - o]
                raise AssertionError((lo, hi))

            def o_col_group(j, h, blocks, last):
                """O columns [1024h, 1024h+1024): PSUM-accumulate the group's
                blocks through a transient slot, then one VectorE add into
                the SBUF accumulator. The last group also borrows the "ta"
                slots (free once the final exps issue) to shorten the tail."""
                if last and h in (1, 2):
                    Ot = trans_pool.tile([D, 1024], F32, tag="ta", bufs=2)
                else:
                    Ot = trans_pool.tile([D, 1024], F32, tag="tot", bufs=1)
                nb = len(blocks)
                for b, (pchunks, Xs) in enumerate(blocks):
                    for m in range(2):
                        lo = h * 1024 + m * 512
                        nc.tensor.matmul(
                            Ot[:, m * 512:(m + 1) * 512], Xs[:],
                            p_slice(pchunks, lo, lo + 512),
                            start=(b == 0), stop=(b == nb - 1),
                            skip_group_check=True)
                dst = O_acc[:, h * 1024:(h + 1) * 1024]
                if j == 0:
                    nc.vector.tensor_copy(dst, Ot[:])
                else:
                    nc.vector.tensor_add(dst, dst, Ot[:])
                if last:
                    eng = nc.sync if h % 2 == 0 else nc.scalar
                    eng.dma_start(y_d[:, h * 1024:(h + 1) * 1024], dst)

            # Emission order interleaves each column-group's O-matmul burst
            # (<=8 MMs) between a_phases so the in-order PE never starves the
            # scalar engine of prepared A-chunks. Final groups taper (2,1,1)
            # to shorten the post-last-exp tail.
            sizes = [GRP] * (NB // GRP)
            assert sum(sizes) == NB
            next_a = 0
            xr_group(0, split=2)
            xr_group(1, split=2)
            xr_group(2)
            for g in range(3, 8):
                a_phase(next_a)
                next_a += 1
                xr_group(g)
            b0 = 0
            for j, sz in enumerate(sizes):
                blocks = [s_chain(i) for i in range(b0, b0 + sz)]
                b0 += sz
                for h in range(4):
                    o_col_group(j, h, blocks, last=(j == len(sizes) - 1))
                    if next_a < NB:
                        a_phase(next_a)
                        next_a += 1

    _split_waits(nc)
    return nc


_NC = None


def _get_nc():
    global _NC
    if _NC is None:
        _NC = _build_nc()
    return _NC


def _in_maps(x):
    return [{"x": np.ascontiguousarray(x[b].reshape(N, D))}
            for b in range(N_CORES)]


def kernel(x):
    x = np.asarray(x)
    assert x.shape == (N_CORES, D, 64, 64), x.shape
    res = run_bass_kernel_spmd(_get_nc(), _in_maps(x),
                               core_ids=list(range(N_CORES)))
    out = np.stack([res.results[b]["y"] for b in range(N_CORES)])
    return out.reshape(N_CORES, D, 64, 64).astype(np.float32)


# revision 31
# speedup vs baseline: 1.0193x; 1.0193x over previous
# Trainium2 Bass kernel for nn_ChannelAttentionBlock:
#   per batch b: F = x[b].reshape(4096, 128)  (raw row-major view)
#                A = F @ F.T            [4096, 4096]
#                P = softmax(A, axis=-1)
#                out[b] = (F.T @ P).reshape(128, 64, 64)
#
# Sharding: data-parallel over batch — B=8 batches, one per NeuronCore.
#
# Per-core algorithm (X := F.T as [128, 4096], n-blocked by 128 rows of A):
#   prologue: DMA F row-blocks (contiguous) -> XT tiles; PE-transpose them
#             into Xr = float32r(F.T); negdiag[n] = -sum_k F[n,k]^2 (softmax
#             shift — any per-row constant is mathematically exact; the
#             diagonal dominates each row of A for this Gram matrix, making
#             exp overflow-safe).
#   per row-block i: A_i = Xr[:,blk_i].T @ Xr (8 matmuls of [128,512] into
#             chunks of [1536,1536,1024]), P~_i = exp(A_i + negdiag_i) via
#             ScalarE with per-partition bias and accum_out giving the row
#             sums; fold 1/s into the O-matmul's stationary operand:
#             Xs_i = XT_i * (1/s_i); O += Xs_i.T @ P~_i, PSUM-accumulated
#             over groups of 4 blocks through one transient 2-bank slot,
#             then VectorE-added into an SBUF accumulator.
#
# The loop is software-pipelined for the ScalarE exp stream (the bottleneck
# engine at ~145us busy): A-chunks own two dedicated 3-bank PSUM slots, and
# each O column-group burst is emitted between a_phases so the in-order PE
# never starves ACT. TimelineSim predicts ~168us/core; ScalarE ~86% busy.
#
# Matmuls run in float32r (TF32-like, 1 cycle/row at N=512 — 4x faster than
# plain fp32, ~16x more accurate than bf16; measured rel err ~1.5e-4).

import numpy as np

import concourse.bass as bass
import concourse.mybir as mybir
import concourse.tile as tile
from concourse.bass_utils import run_bass_kernel_spmd

N_CORES = 8
D = 128          # feature dim / partition dim
N = 4096         # sequence dim (64*64)
NB = N // 128    # 32 row blocks
F32 = mybir.dt.float32
F32R = mybir.dt.float32r
AX = mybir.AxisListType
ALU = mybir.AluOpType
ACT = mybir.ActivationFunctionType


def _split_waits(nc, max_waits=1):
    """walrus in this toolchain encodes at most 1 semaphore wait per
    instruction; Tile emits several on its tail drain. Move overflow waits
    onto preceding same-engine NoOps (sequencer executes them in order)."""
    n_split = 0
    for f in nc.m.functions:
        for bb in f.blocks:
            new_insts = []
            for inst in bb.instructions:
                si = inst.sync_info
                if si is not None and si.on_wait and len(si.on_wait) > max_waits:
                    waits = list(si.on_wait)
                    chunks = [waits[i:i + max_waits]
                              for i in range(0, len(waits), max_waits)]
                    for chunk in chunks[:-1]:
                        nop = mybir.InstNoOp(
                            name=nc.get_next_instruction_name(), ins=[], outs=[])
                        nop.engine = inst.engine
                        nop.sync_info = mybir.SyncInfo(on_wait=chunk, on_update=[])
                        new_insts.append(nop)
                        n_split += 1
                    inst.sync_info = mybir.SyncInfo(
                        on_wait=chunks[-1],
                        on_update=list(si.on_update) if si.on_update else [])
                new_insts.append(inst)
            bb.instructions = new_insts
    return n_split


def _build_nc():
    nc = bass.Bass("TRN2", target_bir_lowering=False, debug=False)
    x_d = nc.dram_tensor("x", [N, D], F32, kind="ExternalInput").ap()
    y_d = nc.dram_tensor("y", [D, N], F32, kind="ExternalOutput").ap()

    # A-row chunk widths per block (sum = N). Fewer/bigger chunks amortize
    # the ScalarE per-instruction overhead (~404ns: access-latency + accum
    # read + dispatch) over more exp elements.
    CH = [1536, 1536, 1024]
    CO = [0, 1536, 3072]          # chunk column offsets
    GRP = 4                       # blocks per O-accumulation group

    with tile.TileContext(nc) as tc:
        with tc.tile_pool(name="const", bufs=1) as const, \
             tc.tile_pool(name="ppool", bufs=1) as ppool, \
             tc.tile_pool(name="loop", bufs=12) as loop, \
             tc.tile_pool(name="trans", bufs=1, space="PSUM") as trans_pool:

            XT = const.tile([D, N], F32, tag="XT")      # XT[:,128i:..] = F[blk_i,:]
            Xr = const.tile([D, N], F32R, tag="Xr")     # rounded F.T
            O_acc = const.tile([D, N], F32, tag="Oacc") # SBUF output accumulator
            negdiag = const.tile([D, NB], F32, tag="negdiag")
            ident = const.tile([D, D], F32, tag="ident")

            # XT[p, 128i+k] = x_d[128i+p, k]; contiguous 512B bursts per row.
            # The transpose identity is built on-chip (memset + affine_select)
            # so the HWDGE dispatch queue only carries the x loads.
            nc.gpsimd.memset(ident[:], 1.0)
            nc.gpsimd.affine_select(ident[:], ident[:], [[1, D]],
                                    ALU.is_equal, 0.0, base=0,
                                    channel_multiplier=-1)
            # Input DMAs split across both HWDGE queues (SP + ACT
            # sequencers) to halve the serialized queue-dispatch time.
            x_r = x_d.rearrange("(i p) k -> p i k", p=D)
            XT_v = XT[:].rearrange("p (i k) -> p i k", k=D)
            for g in range(8):
                eng = nc.sync if g % 2 == 0 else nc.scalar
                eng.dma_start(XT_v[:, g * 4:(g + 1) * 4, :],
                              x_r[:, g * 4:(g + 1) * 4, :])

            # Prologue: Xr = f32r(F.T) via PE transposes, 4 per PSUM tile,
            # one VectorE evacuation copy each (kept off ScalarE so the
            # in-order exp stream is not stalled behind prologue copies).
            def xr_group(g, split=1):
                for v in range(split):
                    w = 512 // split
                    tp = trans_pool.tile([D, w], F32, tag="ta", bufs=2)
                    for u in range(w // D):
                        i = (g * 512 + v * w) // D + u
                        nc.tensor.transpose(tp[:, u * D:(u + 1) * D],
                                            XT[:, i * D:(i + 1) * D], ident[:])
                    nc.vector.tensor_copy(
                        Xr[:, g * 512 + v * w:g * 512 + (v + 1) * w], tp[:])

            # Software-pipelined main loop. A-chunks own two dedicated
            # 3-bank PSUM slots (tag "ta"); all O columns flow through ONE
            # transient 2-bank slot (tag "tot") that PSUM-accumulates GRP
            # blocks per VectorE add into the SBUF accumulator.
            state = {}

            def a_phase(i):
                lhsA = Xr[:, i * D:(i + 1) * D]
                # negdiag[p, i] = -sum_k F[128i+p, k]^2 (= -A[n,n], n=128i+p)
                nsq = loop.tile([D, D], F32, tag="nsq")
                nc.vector.scalar_tensor_tensor(
                    nsq[:], XT[:, i * D:(i + 1) * D], -1.0,
                    XT[:, i * D:(i + 1) * D], op0=ALU.mult, op1=ALU.mult)
                nc.vector.tensor_reduce(negdiag[:, i:i + 1], nsq[:],
                                        axis=AX.X, op=ALU.add)
                spart = loop.tile([D, 3], F32, tag="spart")
                pchunks = []
                for c, (w, o) in enumerate(zip(CH, CO)):
                    At = trans_pool.tile([D, w], F32, tag="ta", bufs=2)
                    for m in range(w // 512):
                        nc.tensor.matmul(
                            At[:, m * 512:(m + 1) * 512], lhsA,
                            Xr[:, o + m * 512:o + (m + 1) * 512],
                            start=True, stop=True)
                    P_c = ppool.tile([D, w], F32R, tag=f"p{c}", bufs=9)
                    if c < 2:
                        nc.scalar.activation(P_c[:], At[:], ACT.Exp,
                                             bias=negdiag[:, i:i + 1],
                                             accum_out=spart[:, c:c + 1])
                    else:
                        # last chunk: skip the ScalarE accumulator read
                        # (187ns/instr on the bottleneck engine); DVE has
                        # slack and computes this partial sum instead.
                        nc.scalar.activation(P_c[:], At[:], ACT.Exp,
                                             bias=negdiag[:, i:i + 1])
                        nc.vector.tensor_reduce(spart[:, c:c + 1], P_c[:],
                                                axis=AX.X, op=ALU.add)
                    pchunks.append(P_c)
                state[i] = (pchunks, spart)

            def s_chain(i):
                pchunks, spart = state.pop(i)
                s = loop.tile([D, 1], F32, tag="s")
                nc.vector.tensor_reduce(s[:], spart[:], axis=AX.X, op=ALU.add)
                r = loop.tile([D, 1], F32, tag="r")
                nc.vector.reciprocal(r[:], s[:])
                Xs = loop.tile([D, D], F32R, tag="xs")
                nc.gpsimd.tensor_scalar_mul(Xs[:], XT[:, i * D:(i + 1) * D], r[:])
                return pchunks, Xs

            def p_slice(pchunks, lo, hi):
                """AP for O-columns [lo, hi) inside the per-block chunk list."""
                for c, (w, o) in enumerate(zip(CH, CO)):
                    if o <= lo and hi <= o + w:
                        return pchunks[c][:, lo - o:hi - o]
                raise AssertionError((lo, hi))

            def o_col_group(j, h, blocks, last):
                """O columns [1024h, 1024h+1024): PSUM-accumulate the group's
                blocks through a transient slot, then one VectorE add into
                the SBUF accumulator. The last group also borrows the "ta"
                slots (free once the final exps issue) to shorten the tail."""
                if last and h in (1, 2):
                    Ot = trans_pool.tile([D, 1024], F32, tag="ta", bufs=2)
                else:
                    Ot = trans_pool.tile([D, 1024], F32, tag="tot", bufs=1)
                nb = len(blocks)
                for b, (pchunks, Xs) in enumerate(blocks):
                    for m in range(2):
                        lo = h * 1024 + m * 512
                        nc.tensor.matmul(
                            Ot[:, m * 512:(m + 1) * 512], Xs[:],
                            p_slice(pchunks, lo, lo + 512),
                            start=(b == 0), stop=(b == nb - 1),
                            skip_group_check=True)
                dst = O_acc[:, h * 1024:(h + 1) * 1024]
                if j == 0:
                    nc.vector.tensor_copy(dst, Ot[:])
                else:
                    nc.vector.tensor_add(dst, dst, Ot[:])


            # Emission order interleaves each column-group's O-matmul burst
            # (<=8 MMs) between a_phases so the in-order PE never starves the
            # scalar engine of prepared A-chunks. Final groups taper (2,1,1)
            # to shorten the post-last-exp tail.
            next_a = 0
            xr_group(0, split=2)
            xr_group(1, split=2)
            xr_group(2)
            for g in range(3, 8):
                a_phase(next_a)
                next_a += 1
                xr_group(g)
            # Column-group bursts interleave with the remaining a_phases.
            # The last group's O columns are finalized per column-group as
            # soon as block 31's corresponding exp lands, and each group's
            # output DMA follows its final add.
            for j in range(NB // GRP):
                last = (j == NB // GRP - 1)
                blocks = [s_chain(i) for i in range(GRP * j, GRP * (j + 1))]
                for h in range(4):
                    o_col_group(j, h, blocks, last=last)
                    if last:
                        for q in range(2):
                            eng = nc.sync if q == 0 else nc.scalar
                            sl = slice(h * 1024 + q * 512,
                                       h * 1024 + (q + 1) * 512)
                            eng.dma_start(y_d[:, sl], O_acc[:, sl])
                    if next_a < NB:
                        a_phase(next_a)
                        next_a += 1

    _split_waits(nc)
    return nc


_NC = None


def _get_nc():
    global _NC
    if _NC is None:
        _NC = _build_nc()
    return _NC


def _in_maps(x):
    return [{"x": np.ascontiguousarray(x[b].reshape(N, D))}
            for b in range(N_CORES)]


def kernel(x):
    x = np.asarray(x)
    assert x.shape == (N_CORES, D, 64, 64), x.shape
    res = run_bass_kernel_spmd(_get_nc(), _in_maps(x),
                               core_ids=list(range(N_CORES)))
    out = np.stack([res.results[b]["y"] for b in range(N_CORES)])
    return out.reshape(N_CORES, D, 64, 64).astype(np.float32)


# revision 35
# speedup vs baseline: 1.0274x; 1.0079x over previous
# Trainium2 Bass kernel for nn_ChannelAttentionBlock:
#   per batch b: F = x[b].reshape(4096, 128)  (raw row-major view)
#                A = F @ F.T            [4096, 4096]
#                P = softmax(A, axis=-1)
#                out[b] = (F.T @ P).reshape(128, 64, 64)
#
# Sharding: data-parallel over batch — B=8 batches, one per NeuronCore.
#
# Per-core algorithm (X := F.T as [128, 4096], n-blocked by 128 rows of A):
#   prologue: DMA F row-blocks (contiguous) -> XT tiles; PE-transpose them
#             into Xr = float32r(F.T); negdiag[n] = -sum_k F[n,k]^2 (softmax
#             shift — any per-row constant is mathematically exact; the
#             diagonal dominates each row of A for this Gram matrix, making
#             exp overflow-safe).
#   per row-block i: A_i = Xr[:,blk_i].T @ Xr (8 matmuls of [128,512] into
#             chunks of [1536,1536,1024]), P~_i = exp(A_i + negdiag_i) via
#             ScalarE with per-partition bias and accum_out giving the row
#             sums; fold 1/s into the O-matmul's stationary operand:
#             Xs_i = XT_i * (1/s_i); O += Xs_i.T @ P~_i, PSUM-accumulated
#             over groups of 4 blocks through one transient 2-bank slot,
#             then VectorE-added into an SBUF accumulator.
#
# The loop is software-pipelined for the ScalarE exp stream (the bottleneck
# engine at ~139us busy): A-chunks own two dedicated 3-bank PSUM slots, each
# O column-group burst is emitted between a_phases so the in-order PE never
# starves ACT, and the last chunk's softmax row-sum moves to VectorE (slack
# engine) to skip ScalarE's per-instruction accumulator read. TimelineSim
# predicts ~165us/core; ScalarE ~84% busy, PE ~68%, DVE ~57%.
#
# Matmuls run in float32r (TF32-like, 1 cycle/row at N=512 — 4x faster than
# plain fp32, ~16x more accurate than bf16; measured rel err ~1.5e-4).

import numpy as np

import concourse.bass as bass
import concourse.mybir as mybir
import concourse.tile as tile
from concourse.bass_utils import run_bass_kernel_spmd

N_CORES = 8
D = 128          # feature dim / partition dim
N = 4096         # sequence dim (64*64)
NB = N // 128    # 32 row blocks
F32 = mybir.dt.float32
F32R = mybir.dt.float32r
AX = mybir.AxisListType
ALU = mybir.AluOpType
ACT = mybir.ActivationFunctionType


def _split_waits(nc, max_waits=1):
    """walrus in this toolchain encodes at most 1 semaphore wait per
    instruction; Tile emits several on its tail drain. Move overflow waits
    onto preceding same-engine NoOps (sequencer executes them in order)."""
    n_split = 0
    for f in nc.m.functions:
        for bb in f.blocks:
            new_insts = []
            for inst in bb.instructions:
                si = inst.sync_info
                if si is not None and si.on_wait and len(si.on_wait) > max_waits:
                    waits = list(si.on_wait)
                    chunks = [waits[i:i + max_waits]
                              for i in range(0, len(waits), max_waits)]
                    for chunk in chunks[:-1]:
                        nop = mybir.InstNoOp(
                            name=nc.get_next_instruction_name(), ins=[], outs=[])
                        nop.engine = inst.engine
                        nop.sync_info = mybir.SyncInfo(on_wait=chunk, on_update=[])
                        new_insts.append(nop)
                        n_split += 1
                    inst.sync_info = mybir.SyncInfo(
                        on_wait=chunks[-1],
                        on_update=list(si.on_update) if si.on_update else [])
                new_insts.append(inst)
            bb.instructions = new_insts
    return n_split


def _build_nc():
    nc = bass.Bass("TRN2", target_bir_lowering=False, debug=False)
    x_d = nc.dram_tensor("x", [N, D], F32, kind="ExternalInput").ap()
    y_d = nc.dram_tensor("y", [D, N], F32, kind="ExternalOutput").ap()

    # A-row chunk widths per block (sum = N). Fewer/bigger chunks amortize
    # the ScalarE per-instruction overhead (~404ns: access-latency + accum
    # read + dispatch) over more exp elements.
    CH = [1536, 1536, 1024]
    CO = [0, 1536, 3072]          # chunk column offsets
    GRP = 4                       # blocks per O-accumulation group

    with tile.TileContext(nc) as tc:
        with tc.tile_pool(name="const", bufs=1) as const, \
             tc.tile_pool(name="ppool", bufs=1) as ppool, \
             tc.tile_pool(name="loop", bufs=12) as loop, \
             tc.tile_pool(name="trans", bufs=1, space="PSUM") as trans_pool:

            XT = const.tile([D, N], F32, tag="XT")      # XT[:,128i:..] = F[blk_i,:]
            Xr = const.tile([D, N], F32R, tag="Xr")     # rounded F.T
            O_acc = const.tile([D, N], F32, tag="Oacc") # SBUF output accumulator
            negdiag = const.tile([D, NB], F32, tag="negdiag")
            ident = const.tile([D, D], F32, tag="ident")

            # XT[p, 128i+k] = x_d[128i+p, k]; contiguous 512B bursts per row.
            # The transpose identity is built on-chip (memset + affine_select)
            # so the HWDGE dispatch queue only carries the x loads.
            nc.gpsimd.memset(ident[:], 1.0)
            nc.gpsimd.affine_select(ident[:], ident[:], [[1, D]],
                                    ALU.is_equal, 0.0, base=0,
                                    channel_multiplier=-1)
            # Input DMAs split across both HWDGE queues (SP + ACT
            # sequencers) to halve the serialized queue-dispatch time.
            x_r = x_d.rearrange("(i p) k -> p i k", p=D)
            XT_v = XT[:].rearrange("p (i k) -> p i k", k=D)
            for g in range(8):
                eng = nc.sync if g % 2 == 0 else nc.scalar
                eng.dma_start(XT_v[:, g * 4:(g + 1) * 4, :],
                              x_r[:, g * 4:(g + 1) * 4, :])

            # Prologue: Xr = f32r(F.T) via PE transposes, 4 per PSUM tile,
            # one VectorE evacuation copy each (kept off ScalarE so the
            # in-order exp stream is not stalled behind prologue copies).
            def xr_group(g, split=1):
                for v in range(split):
                    w = 512 // split
                    tp = trans_pool.tile([D, w], F32, tag="ta", bufs=2)
                    for u in range(w // D):
                        i = (g * 512 + v * w) // D + u
                        nc.tensor.transpose(tp[:, u * D:(u + 1) * D],
                                            XT[:, i * D:(i + 1) * D], ident[:])
                    nc.vector.tensor_copy(
                        Xr[:, g * 512 + v * w:g * 512 + (v + 1) * w], tp[:])

            # Software-pipelined main loop. A-chunks own two dedicated
            # 3-bank PSUM slots (tag "ta"); all O columns flow through ONE
            # transient 2-bank slot (tag "tot") that PSUM-accumulates GRP
            # blocks per VectorE add into the SBUF accumulator.
            state = {}

            def a_phase(i):
                lhsA = Xr[:, i * D:(i + 1) * D]
                # negdiag[p, i] = -sum_k F[128i+p, k]^2 (= -A[n,n], n=128i+p)
                nsq = loop.tile([D, D], F32, tag="nsq")
                nc.vector.scalar_tensor_tensor(
                    nsq[:], XT[:, i * D:(i + 1) * D], -1.0,
                    XT[:, i * D:(i + 1) * D], op0=ALU.mult, op1=ALU.mult)
                nc.vector.tensor_reduce(negdiag[:, i:i + 1], nsq[:],
                                        axis=AX.X, op=ALU.add)
                spart = loop.tile([D, 3], F32, tag="spart")
                pchunks = []
                for c, (w, o) in enumerate(zip(CH, CO)):
                    At = trans_pool.tile([D, w], F32, tag="ta", bufs=2)
                    for m in range(w // 512):
                        nc.tensor.matmul(
                            At[:, m * 512:(m + 1) * 512], lhsA,
                            Xr[:, o + m * 512:o + (m + 1) * 512],
                            start=True, stop=True)
                    P_c = ppool.tile([D, w], F32R, tag=f"p{c}", bufs=9)
                    if c < 2 or i == NB - 1 or i < 6:
                        nc.scalar.activation(P_c[:], At[:], ACT.Exp,
                                             bias=negdiag[:, i:i + 1],
                                             accum_out=spart[:, c:c + 1])
                    else:
                        # last chunk: skip the ScalarE accumulator read
                        # (187ns/instr on the bottleneck engine); DVE has
                        # slack and computes this partial sum instead.
                        nc.scalar.activation(P_c[:], At[:], ACT.Exp,
                                             bias=negdiag[:, i:i + 1])
                        nc.vector.tensor_reduce(spart[:, c:c + 1], P_c[:],
                                                axis=AX.X, op=ALU.add)
                    pchunks.append(P_c)
                state[i] = (pchunks, spart)

            def s_chain(i):
                pchunks, spart = state.pop(i)
                s = loop.tile([D, 1], F32, tag="s")
                nc.vector.tensor_reduce(s[:], spart[:], axis=AX.X, op=ALU.add)
                r = loop.tile([D, 1], F32, tag="r")
                nc.vector.reciprocal(r[:], s[:])
                Xs = loop.tile([D, D], F32R, tag="xs")
                nc.gpsimd.tensor_scalar_mul(Xs[:], XT[:, i * D:(i + 1) * D], r[:])
                return pchunks, Xs

            def p_slice(pchunks, lo, hi):
                """AP for O-columns [lo, hi) inside the per-block chunk list."""
                for c, (w, o) in enumerate(zip(CH, CO)):
                    if o <= lo and hi <= o + w:
                        return pchunks[c][:, lo - o:hi - o]
                raise AssertionError((lo, hi))

            def o_col_group(j, h, blocks, last):
                """O columns [1024h, 1024h+1024): PSUM-accumulate the group's
                blocks through a transient slot, then one VectorE add into
                the SBUF accumulator. The last group also borrows the "ta"
                slots (free once the final exps issue) to shorten the tail."""
                if last and h in (1, 2):
                    Ot = trans_pool.tile([D, 1024], F32, tag="ta", bufs=2)
                else:
                    Ot = trans_pool.tile([D, 1024], F32, tag="tot", bufs=1)
                nb = len(blocks)
                for b, (pchunks, Xs) in enumerate(blocks):
                    for m in range(2):
                        lo = h * 1024 + m * 512
                        nc.tensor.matmul(
                            Ot[:, m * 512:(m + 1) * 512], Xs[:],
                            p_slice(pchunks, lo, lo + 512),
                            start=(b == 0), stop=(b == nb - 1),
                            skip_group_check=True)
                dst = O_acc[:, h * 1024:(h + 1) * 1024]
                if j == 0:
                    nc.vector.tensor_copy(dst, Ot[:])
                else:
                    nc.vector.tensor_add(dst, dst, Ot[:])


            # Emission order interleaves each column-group's O-matmul burst
            # (<=8 MMs) between a_phases so the in-order PE never starves the
            # scalar engine of prepared A-chunks. Final groups taper (2,1,1)
            # to shorten the post-last-exp tail.
            next_a = 0
            xr_group(0, split=2)
            xr_group(1, split=2)
            xr_group(2)
            for g in range(3, 8):
                a_phase(next_a)
                next_a += 1
                xr_group(g)
            # Column-group bursts interleave with the remaining a_phases.
            # The last group's O columns are finalized per column-group as
            # soon as block 31's corresponding exp lands, and each group's
            # output DMA follows its final add.
            for j in range(NB // GRP):
                last = (j == NB // GRP - 1)
                blocks = [s_chain(i) for i in range(GRP * j, GRP * (j + 1))]
                for h in range(4):
                    o_col_group(j, h, blocks, last=last)
                    if last:
                        for q in range(2):
                            eng = nc.sync if q == 0 else nc.scalar
                            sl = slice(h * 1024 + q * 512,
                                       h * 1024 + (q + 1) * 512)
                            eng.dma_start(y_d[:, sl], O_acc[:, sl])
                    if next_a < NB:
                        a_phase(next_a)
                        next_a += 1

    _split_waits(nc)
    return nc


_NC = None


def _get_nc():
    global _NC
    if _NC is None:
        _NC = _build_nc()
    return _NC


def _in_maps(x):
    return [{"x": np.ascontiguousarray(x[b].reshape(N, D))}
            for b in range(N_CORES)]


def kernel(x):
    x = np.asarray(x)
    assert x.shape == (N_CORES, D, 64, 64), x.shape
    res = run_bass_kernel_spmd(_get_nc(), _in_maps(x),
                               core_ids=list(range(N_CORES)))
    out = np.stack([res.results[b]["y"] for b in range(N_CORES)])
    return out.reshape(N_CORES, D, 64, 64).astype(np.float32)
